# revision 1
# baseline (speedup 1.0000x reference)
"""MoE expert-choice routing kernel for 8 TRN2 NeuronCores.

Strategy (expert-parallel, one expert per core):
  host: routing in float64 (logits -> softmax -> top-512 tokens per
        (batch, expert)), gather of selected token rows, operand
        pre-transpose + bf16 pre-cast.
  device (per core, expert e): y = silu(xin @ w1[e].T) @ w2[e].T scaled
        by the gates; two chained matmuls with the hidden activations
        kept in SBUF.
  host: scatter-add of the 8 per-expert partial outputs (token indices
        are unique within one (batch, expert) pair).
"""
import sys

if "/opt/trn_rl_repo" not in sys.path:
    sys.path.insert(0, "/opt/trn_rl_repo")

import numpy as np
import ml_dtypes

B = 4          # batch
S = 2048       # tokens per batch (block size)
D = 1024       # d_model
F = 4096       # d_ffn
E = 8          # experts == cores
K = 512        # tokens per (batch, expert)
T = B * K      # 2048 token rows per core
P = 128
TB = 512       # token block in the device kernel
NB = T // TB   # 4
DT = D // P    # 8
FT = F // P    # 32

_NC = None
_NAMES = None


def _build():
    """Build + compile the per-core Bass program once."""
    global _NC, _NAMES
    if _NC is not None:
        return _NC, _NAMES

    import concourse.mybir as mybir
    import concourse.tile as tile
    from concourse import bacc

    BF = mybir.dt.bfloat16
    F32 = mybir.dt.float32

    nc = bacc.Bacc(None, target_bir_lowering=False)
    with tile.TileContext(nc) as tc:
        with tc.tile_pool(name="dram", bufs=1, space="DRAM") as dram:
            FC = 512  # w1 f-chunk: ft=0..3 chains only need chunk 0
            xinT = dram.tile([D, T], BF, kind="ExternalInput", name="xinT")
            w1T = dram.tile([F // FC, D, FC], BF, kind="ExternalInput", name="w1T")
            w2T = dram.tile([F, D], BF, kind="ExternalInput", name="w2T")
            g = dram.tile([P, T // P], F32, kind="ExternalInput", name="g")
            y = dram.tile([T, D], F32, kind="ExternalOutput", name="y")

            with (
                tc.tile_pool(name="wpool", bufs=1) as wpool,
                tc.tile_pool(name="xpool", bufs=2) as xpool,
                tc.tile_pool(name="hpool", bufs=1) as hpool,
                tc.tile_pool(name="ps1", bufs=2, space="PSUM") as ps1pool,
                tc.tile_pool(name="ps2", bufs=3, space="PSUM") as ps2pool,
                tc.tile_pool(name="ypool", bufs=4) as ypool,
            ):
                w1s = wpool.tile([P, DT, F], BF, name="w1s")
                w2s = wpool.tile([P, FT, D], BF, name="w2s")
                gs = wpool.tile([P, T // P], F32, name="gs")
                nc.sync.dma_start(gs[:], g[:])
                # critical-path order on one queue (HBM-bound anyway):
                # block-0 activations, then w1 f-chunk-major (each chunk DMA
                # is one contiguous 256KB region in the blocked host layout,
                # and the ft=0..7 chains only need chunk 0), then w2 (first
                # needed ~60us in, in ft order).
                # HAM pre-warm: ~12 zero matmuls keep the PE busy through one
                # activity window during the startup DMAs, so the real
                # matmuls start at 2.4GHz instead of ramping at 1.2GHz.
                warm_w = wpool.tile([P, P], BF, name="warm_w")
                warm_x = wpool.tile([P, TB], BF, name="warm_x")
                nc.vector.memset(warm_w[:], 0)
                nc.vector.memset(warm_x[:], 0)
                ps_warm = ps1pool.tile([P, TB], F32, name="ps1")
                NWARM = 32
                for i in range(NWARM):
                    nc.tensor.matmul(
                        ps_warm[:, 0:P], warm_w[:], warm_x[:, 0:P],
                        start=(i == 0), stop=(i == NWARM - 1),
                    )

                xs0 = xpool.tile([P, DT, TB], BF, name="xs")
                for dt in range(DT):
                    nc.sync.dma_start(xs0[:, dt, :], xinT[dt * P:(dt + 1) * P, 0:TB])
                    nc.sync.dma_start(
                        w1s[:, dt, 0:FC], w1T[0, dt * P:(dt + 1) * P, :]
                    )
                for fc in range(1, F // FC):
                    for dt in range(DT):
                        nc.sync.dma_start(
                            w1s[:, dt, fc * FC:(fc + 1) * FC],
                            w1T[fc, dt * P:(dt + 1) * P, :],
                        )
                for ft in range(FT):
                    nc.sync.dma_start(w2s[:, ft, :], w2T[ft * P:(ft + 1) * P, :])

                for tb in range(NB):
                    if tb == 0:
                        xs = xs0
                    else:
                        xs = xpool.tile([P, DT, TB], BF, name="xs")
                        for dt in range(DT):
                            nc.sync.dma_start(
                                xs[:, dt, :],
                                xinT[dt * P:(dt + 1) * P, tb * TB:(tb + 1) * TB],
                            )
                    # mm1: hT[f, t] = silu(w1T.T @ xinT) for this token block
                    hs = hpool.tile([P, FT, TB], BF, name="hs")
                    for ft in range(FT):
                        ps = ps1pool.tile([P, TB], F32, name="ps1")
                        for dt in range(DT):
                            nc.tensor.matmul(
                                ps[:],
                                w1s[:, dt, ft * P:(ft + 1) * P],
                                xs[:, dt, :],
                                start=(dt == 0),
                                stop=(dt == DT - 1),
                            )
                        nc.scalar.activation(
                            hs[:, ft, :], ps[:],
                            mybir.ActivationFunctionType.Silu,
                        )
                    # mm2: y[t, d] = hT.T @ w2T, scaled per-token by gates
                    for tt in range(TB // P):
                        col = tb * (TB // P) + tt
                        ps2 = [
                            ps2pool.tile([P, 512], F32, name=f"ps2_{dc}")
                            for dc in range(D // 512)
                        ]
                        for ft in range(FT):
                            for dc in range(D // 512):
                                nc.tensor.matmul(
                                    ps2[dc][:],
                                    hs[:, ft, tt * P:(tt + 1) * P],
                                    w2s[:, ft, dc * 512:(dc + 1) * 512],
                                    start=(ft == 0),
                                    stop=(ft == FT - 1),
                                )
                        for dc in range(D // 512):
                            ys = ypool.tile([P, 512], F32, name=f"ys_{dc}")
                            nc.vector.tensor_scalar_mul(
                                ys[:], ps2[dc][:], gs[:, col:col + 1]
                            )
                            nc.sync.dma_start(
                                y[col * P:(col + 1) * P, dc * 512:(dc + 1) * 512],
                                ys[:],
                            )
    nc.compile()
    _NC = nc
    _NAMES = (xinT.name, w1T.name, w2T.name, g.name, y.name)
    return _NC, _NAMES


def _to_bf16(a):
    """Fast f32 -> bf16 with round-to-nearest-even."""
    a = np.ascontiguousarray(a, dtype=np.float32)
    v = a.view(np.uint32)
    r = ((v + np.uint32(0x7FFF) + ((v >> np.uint32(16)) & np.uint32(1)))
         >> np.uint32(16)).astype(np.uint16)
    return r.view(ml_dtypes.bfloat16)


def _routing(x, choice):
    """float64 routing: per (batch, expert) top-K token ids + gates."""
    logits = np.einsum(
        "bsd,ed->bse",
        x.astype(np.float64), choice.astype(np.float64),
        optimize=True,
    )
    m = logits.max(axis=-1, keepdims=True)
    p = np.exp(logits - m)
    probs = p / p.sum(axis=-1, keepdims=True)  # [b, s, e]
    idx = np.empty((B, E, K), dtype=np.int64)
    gates = np.empty((B, E, K), dtype=np.float32)
    for b in range(B):
        for e in range(E):
            pe = probs[b, :, e]
            ii = np.argpartition(-pe, K)[:K]
            ii = np.sort(ii)
            idx[b, e] = ii
            gates[b, e] = pe[ii].astype(np.float32)
    return idx, gates


def kernel(x, choice, w1, w2):
    from concourse.bass_utils import run_bass_kernel_spmd

    x = np.ascontiguousarray(x, dtype=np.float32)
    choice = np.ascontiguousarray(choice, dtype=np.float32)
    w1 = np.ascontiguousarray(w1, dtype=np.float32)
    w2 = np.ascontiguousarray(w2, dtype=np.float32)
    assert x.shape == (B, S, D) and w1.shape == (E, F, D) and w2.shape == (E, D, F)

    nc, (n_xinT, n_w1T, n_w2T, n_g, n_y) = _build()

    idx, gates = _routing(x, choice)

    def _prep(e):
        xin = np.empty((T, D), dtype=np.float32)
        for b in range(B):
            xin[b * K:(b + 1) * K] = x[b, idx[b, e], :]
        FC = 512
        xinT = np.ascontiguousarray(_to_bf16(xin).T)          # [D, T]
        w1T = np.ascontiguousarray(                           # [F//FC, D, FC]
            _to_bf16(w1[e]).T.reshape(D, F // FC, FC).transpose(1, 0, 2)
        )
        w2T = np.ascontiguousarray(_to_bf16(w2[e]).T)         # [F, D]
        gflat = gates[:, e].reshape(T)                        # rows b*K + k
        gcols = np.ascontiguousarray(gflat.reshape(T // P, P).T)  # [P, T//P]
        return {n_xinT: xinT, n_w1T: w1T, n_w2T: w2T, n_g: gcols}

    from concurrent.futures import ThreadPoolExecutor

    with ThreadPoolExecutor(E) as pool:
        in_maps = list(pool.map(_prep, range(E)))

    res = run_bass_kernel_spmd(nc, in_maps, core_ids=list(range(E)))

    out = np.zeros((B, S, D), dtype=np.float32)
    for e in range(E):
        ye = res.results[e][n_y]  # [T, D]
        for b in range(B):
            out[b, idx[b, e], :] += ye[b * K:(b + 1) * K]
    return out



# revision 11
# speedup vs baseline: 1.0368x; 1.0368x over previous
"""MoE expert-choice routing kernel for 8 TRN2 NeuronCores.

Strategy (expert-parallel, one expert per core):
  host: routing in float64 (logits -> softmax -> top-512 tokens per
        (batch, expert)), gather of selected token rows, operand
        pre-transpose + bf16/e4m3 pre-cast.
  device (per core, expert e): y = silu(xin @ w1[e].T) @ w2[e].T scaled
        by the gates; two chained matmuls with the hidden activations
        kept in SBUF.
  host: scatter-add of the 8 per-expert partial outputs (token indices
        are unique within one (batch, expert) pair).

Perf notes vs the plain-bf16 version (468.6us):
  - mm1 does the last 256 of 1024 contraction dims as ONE fp8e4
    DoubleRow matmul (2 fp8 weights per PE cell, contraction 256/pass)
    instead of two bf16 matmuls: 7 instead of 8 PE passes per
    (token-block, ft).  All w1 is pre-scaled by 2^12 (exact in bf16;
    lifts the fp8 slice out of e4m3's denormal range) and the silu
    activation descales with scale=2^-12.  Error budget: e4m3 on 25%
    of mm1's contraction adds ~1.6e-2 rel err (gate is 2e-2).
  - mm2 stays pure bf16 but runs dc-major so the first half of each
    output tile drains while the second half computes.
  - y is written as bf16 (host upcasts), halving output DMA; output
    DMAs issue from the (idle) gpsimd queue.
"""
import sys

if "/opt/trn_rl_repo" not in sys.path:
    sys.path.insert(0, "/opt/trn_rl_repo")

import numpy as np
import ml_dtypes

B = 4          # batch
S = 2048       # tokens per batch (block size)
D = 1024       # d_model
F = 4096       # d_ffn
E = 8          # experts == cores
K = 512        # tokens per (batch, expert)
T = B * K      # 2048 token rows per core
P = 128
TB = 512       # token block in the device kernel
NB = T // TB   # 4
JBF = 6        # bf16 d-blocks of mm1 contraction (dims 0..767)
FT = F // P    # 32
FC = 512       # w1 f-chunk for DMA granularity

_NC = None
_NAMES = None


def _build():
    """Build + compile the per-core Bass program once."""
    global _NC, _NAMES
    if _NC is not None:
        return _NC, _NAMES

    import concourse.mybir as mybir
    import concourse.tile as tile
    from concourse import bacc

    BF = mybir.dt.bfloat16
    F32 = mybir.dt.float32
    F8 = mybir.dt.float8e4
    DR = mybir.MatmulPerfMode.DoubleRow

    nc = bacc.Bacc(None, target_bir_lowering=False)
    with tile.TileContext(nc) as tc:
        with tc.tile_pool(name="dram", bufs=1, space="DRAM") as dram:
            xbf = dram.tile([JBF, P, T], BF, kind="ExternalInput", name="xbf")
            xf8 = dram.tile([2, P, T], F8, kind="ExternalInput", name="xf8")
            w1bf = dram.tile([F // FC, JBF, P, FC], BF, kind="ExternalInput", name="w1bf")
            w1f8 = dram.tile([F // FC, 2, P, FC], F8, kind="ExternalInput", name="w1f8")
            w2bf = dram.tile([FT, P, D], BF, kind="ExternalInput", name="w2bf")
            g = dram.tile([P, T // P], F32, kind="ExternalInput", name="g")
            y = dram.tile([T, D], BF, kind="ExternalOutput", name="y")

            with (
                tc.tile_pool(name="wpool", bufs=1) as wpool,
                tc.tile_pool(name="xpool", bufs=2) as xpool,
                tc.tile_pool(name="hpool", bufs=1) as hpool,
                tc.tile_pool(name="ps1", bufs=2, space="PSUM") as ps1pool,
                tc.tile_pool(name="ps2", bufs=3, space="PSUM") as ps2pool,
                tc.tile_pool(name="ypool", bufs=4) as ypool,
            ):
                w1s = wpool.tile([P, JBF, F], BF, name="w1s")
                w1s8 = wpool.tile([P, 2, F], F8, name="w1s8")
                w2s = wpool.tile([P, FT, D], BF, name="w2s")
                gs = wpool.tile([P, T // P], F32, name="gs")
                nc.sync.dma_start(gs[:], g[:])
                # HAM pre-warm: ~32 zero matmuls keep the PE busy through
                # the startup DMAs so the real matmuls start at 2.4GHz.
                warm_w = wpool.tile([P, P], BF, name="warm_w")
                warm_x = wpool.tile([P, TB], BF, name="warm_x")
                nc.vector.memset(warm_w[:], 0)
                nc.vector.memset(warm_x[:], 0)
                ps_warm = ps1pool.tile([P, TB], F32, name="ps1")
                NWARM = 32
                for i in range(NWARM):
                    nc.tensor.matmul(
                        ps_warm[:, 0:P], warm_w[:], warm_x[:, 0:P],
                        start=(i == 0), stop=(i == NWARM - 1),
                    )

                # critical-path DMA order on the sync queue: block-0
                # activations + first w1 chunk interleaved, then the rest
                # of w1 f-chunk-major, then w2 (first needed ~60us in).
                xs0 = xpool.tile([P, JBF, TB], BF, name="xs")
                xs80 = xpool.tile([P, 2, TB], F8, name="xs8")
                for j in range(JBF):
                    nc.sync.dma_start(xs0[:, j, :], xbf[j, :, 0:TB])
                    nc.sync.dma_start(w1s[:, j, 0:FC], w1bf[0, j, :, :])
                for j in range(2):
                    nc.sync.dma_start(xs80[:, j, :], xf8[j, :, 0:TB])
                    nc.sync.dma_start(w1s8[:, j, 0:FC], w1f8[0, j, :, :])
                for fc in range(1, F // FC):
                    for j in range(JBF):
                        nc.sync.dma_start(
                            w1s[:, j, fc * FC:(fc + 1) * FC], w1bf[fc, j, :, :]
                        )
                    for j in range(2):
                        nc.sync.dma_start(
                            w1s8[:, j, fc * FC:(fc + 1) * FC], w1f8[fc, j, :, :]
                        )
                for ft in range(FT):
                    nc.sync.dma_start(w2s[:, ft, :], w2bf[ft, :, :])

                for tb in range(NB):
                    if tb == 0:
                        xs, xs8 = xs0, xs80
                    else:
                        xs = xpool.tile([P, JBF, TB], BF, name="xs")
                        xs8 = xpool.tile([P, 2, TB], F8, name="xs8")
                        for j in range(JBF):
                            nc.sync.dma_start(
                                xs[:, j, :], xbf[j, :, tb * TB:(tb + 1) * TB]
                            )
                        for j in range(2):
                            nc.sync.dma_start(
                                xs8[:, j, :], xf8[j, :, tb * TB:(tb + 1) * TB]
                            )
                    # mm1: hT[f, t] = silu(2^-12 * (w1*2^12).T @ xinT)
                    hs = hpool.tile([P, FT, TB], BF, name="hs")
                    for ft in range(FT):
                        ps = ps1pool.tile([P, TB], F32, name="ps1")
                        for j in range(JBF):
                            nc.tensor.matmul(
                                ps[:],
                                w1s[:, j, ft * P:(ft + 1) * P],
                                xs[:, j, :],
                                start=(j == 0),
                                stop=False,
                            )
                        nc.tensor.matmul(
                            ps[:],
                            w1s8[:, 0:2, ft * P:(ft + 1) * P],
                            xs8[:, 0:2, :],
                            start=False,
                            stop=True,
                            perf_mode=DR,
                        )
                        nc.scalar.activation(
                            hs[:, ft, :], ps[:],
                            mybir.ActivationFunctionType.Silu,
                            scale=2.0 ** -12,
                        )
                    # mm2: y[t, d] = hT.T @ (w2*2^12).T, gates carry 2^-12
                    for tt in range(TB // P):
                        col = tb * (TB // P) + tt
                        for dc in range(D // 512):
                            ps2 = ps2pool.tile([P, 512], F32, name="ps2")
                            for ft in range(FT):
                                nc.tensor.matmul(
                                    ps2[:],
                                    hs[:, ft, tt * P:(tt + 1) * P],
                                    w2s[:, ft, dc * 512:(dc + 1) * 512],
                                    start=(ft == 0),
                                    stop=(ft == FT - 1),
                                )
                            ys = ypool.tile([P, 512], BF, name="ys")
                            nc.vector.tensor_scalar_mul(
                                ys[:], ps2[:], gs[:, col:col + 1]
                            )
                            nc.gpsimd.dma_start(
                                y[col * P:(col + 1) * P, dc * 512:(dc + 1) * 512],
                                ys[:],
                            )
    nc.compile()
    _NC = nc
    _NAMES = (
        xbf.name, xf8.name, w1bf.name, w1f8.name,
        w2bf.name, g.name, y.name,
    )
    return _NC, _NAMES


def _to_bf16(a):
    """Fast f32 -> bf16 with round-to-nearest-even."""
    a = np.ascontiguousarray(a, dtype=np.float32)
    v = a.view(np.uint32)
    r = ((v + np.uint32(0x7FFF) + ((v >> np.uint32(16)) & np.uint32(1)))
         >> np.uint32(16)).astype(np.uint16)
    return r.view(ml_dtypes.bfloat16)


def _to_e4m3(a):
    """f32 -> TRN fp8_e4m3 (IEEE-style; clip to +-240 to stay finite)."""
    a = np.clip(np.ascontiguousarray(a, dtype=np.float32), -240.0, 240.0)
    return a.astype(ml_dtypes.float8_e4m3)


def _routing(x, choice):
    """float64 routing: per (batch, expert) top-K token ids + gates."""
    logits = np.einsum(
        "bsd,ed->bse",
        x.astype(np.float64), choice.astype(np.float64),
        optimize=True,
    )
    m = logits.max(axis=-1, keepdims=True)
    p = np.exp(logits - m)
    probs = p / p.sum(axis=-1, keepdims=True)  # [b, s, e]
    idx = np.empty((B, E, K), dtype=np.int64)
    gates = np.empty((B, E, K), dtype=np.float32)
    for b in range(B):
        for e in range(E):
            pe = probs[b, :, e]
            ii = np.argpartition(-pe, K)[:K]
            ii = np.sort(ii)
            idx[b, e] = ii
            gates[b, e] = pe[ii].astype(np.float32)
    return idx, gates


def kernel(x, choice, w1, w2):
    from concourse.bass_utils import run_bass_kernel_spmd

    x = np.ascontiguousarray(x, dtype=np.float32)
    choice = np.ascontiguousarray(choice, dtype=np.float32)
    w1 = np.ascontiguousarray(w1, dtype=np.float32)
    w2 = np.ascontiguousarray(w2, dtype=np.float32)
    assert x.shape == (B, S, D) and w1.shape == (E, F, D) and w2.shape == (E, D, F)

    nc, names = _build()
    (n_xbf, n_xf8, n_w1bf, n_w1f8, n_w2bf, n_g, n_y) = names

    idx, gates = _routing(x, choice)
    DBF = JBF * P  # 768

    def _prep(e):
        xin = np.empty((T, D), dtype=np.float32)
        for b in range(B):
            xin[b * K:(b + 1) * K] = x[b, idx[b, e], :]
        xinT = np.ascontiguousarray(xin.T)                    # [D, T]
        xbf = np.ascontiguousarray(
            _to_bf16(xinT[:DBF]).reshape(JBF, P, T))
        xf8 = np.ascontiguousarray(
            _to_e4m3(xinT[DBF:]).reshape(2, P, T))
        w1t = np.ascontiguousarray(w1[e].T) * np.float32(4096.0)  # [D, F]
        w1bf = np.ascontiguousarray(
            _to_bf16(w1t[:DBF]).reshape(JBF, P, F // FC, FC)
            .transpose(2, 0, 1, 3))                           # [8, 6, 128, 512]
        w1f8 = np.ascontiguousarray(
            _to_e4m3(w1t[DBF:]).reshape(2, P, F // FC, FC)
            .transpose(2, 0, 1, 3))                           # [8, 2, 128, 512]
        w2bf = np.ascontiguousarray(
            _to_bf16(np.ascontiguousarray(w2[e].T)).reshape(FT, P, D))
        gflat = gates[:, e].reshape(T)
        gcols = np.ascontiguousarray(gflat.reshape(T // P, P).T)  # [P, T//P]
        return {
            n_xbf: xbf, n_xf8: xf8, n_w1bf: w1bf, n_w1f8: w1f8,
            n_w2bf: w2bf, n_g: gcols,
        }

    from concurrent.futures import ThreadPoolExecutor

    with ThreadPoolExecutor(E) as pool:
        in_maps = list(pool.map(_prep, range(E)))

    res = run_bass_kernel_spmd(nc, in_maps, core_ids=list(range(E)))

    out = np.zeros((B, S, D), dtype=np.float32)
    for e in range(E):
        ye = np.asarray(res.results[e][n_y]).astype(np.float32)  # [T, D]
        for b in range(B):
            out[b, idx[b, e], :] += ye[b * K:(b + 1) * K]
    return out


# revision 14
# speedup vs baseline: 1.0449x; 1.0078x over previous
"""MoE expert-choice routing kernel for 8 TRN2 NeuronCores.

Strategy (expert-parallel, one expert per core):
  host: routing in float64 (logits -> softmax -> top-512 tokens per
        (batch, expert)), gather of selected token rows, operand
        pre-transpose + bf16/e4m3 pre-cast.
  device (per core, expert e): y = silu(xin @ w1[e].T) @ w2[e].T scaled
        by the gates; two chained matmuls with the hidden activations
        kept in SBUF.
  host: scatter-add of the 8 per-expert partial outputs (token indices
        are unique within one (batch, expert) pair).

Perf notes vs the plain-bf16 version (468.6us):
  - mm1 does the last 256 of 1024 contraction dims as ONE fp8e4
    DoubleRow matmul (2 fp8 weights per PE cell, contraction 256/pass)
    instead of two bf16 matmuls: 7 instead of 8 PE passes per
    (token-block, ft).  All w1 is pre-scaled by 2^12 (exact in bf16;
    lifts the fp8 slice out of e4m3's denormal range) and the silu
    activation descales with scale=2^-12.  Error budget: e4m3 on 25%
    of mm1's contraction adds ~1.6e-2 rel err (gate is 2e-2).
  - mm2 stays pure bf16 but runs dc-major so the first half of each
    output tile drains while the second half computes.
  - y is written as bf16 (host upcasts), halving output DMA; output
    DMAs issue from the (idle) gpsimd queue.
"""
import sys

if "/opt/trn_rl_repo" not in sys.path:
    sys.path.insert(0, "/opt/trn_rl_repo")

import numpy as np
import ml_dtypes

B = 4          # batch
S = 2048       # tokens per batch (block size)
D = 1024       # d_model
F = 4096       # d_ffn
E = 8          # experts == cores
K = 512        # tokens per (batch, expert)
T = B * K      # 2048 token rows per core
P = 128
TB = 512       # token block in the device kernel
NB = T // TB   # 4
JBF = 6        # bf16 d-blocks of mm1 contraction (dims 0..767)
FT = F // P    # 32
FC = 512       # w1 f-chunk for DMA granularity

_NC = None
_NAMES = None


def _build():
    """Build + compile the per-core Bass program once."""
    global _NC, _NAMES
    if _NC is not None:
        return _NC, _NAMES

    import concourse.mybir as mybir
    import concourse.tile as tile
    from concourse import bacc

    BF = mybir.dt.bfloat16
    F32 = mybir.dt.float32
    F8 = mybir.dt.float8e4
    DR = mybir.MatmulPerfMode.DoubleRow

    nc = bacc.Bacc(None, target_bir_lowering=False)
    with tile.TileContext(nc) as tc:
        with tc.tile_pool(name="dram", bufs=1, space="DRAM") as dram:
            xbf = dram.tile([JBF, P, T], BF, kind="ExternalInput", name="xbf")
            xf8 = dram.tile([2, P, T], F8, kind="ExternalInput", name="xf8")
            w1bf = dram.tile([F // FC, JBF, P, FC], BF, kind="ExternalInput", name="w1bf")
            w1f8 = dram.tile([F // FC, 2, P, FC], F8, kind="ExternalInput", name="w1f8")
            w2bf = dram.tile([FT, P, D], BF, kind="ExternalInput", name="w2bf")
            g = dram.tile([P, T // P], F32, kind="ExternalInput", name="g")
            y = dram.tile([T, D], BF, kind="ExternalOutput", name="y")

            with (
                tc.tile_pool(name="wpool", bufs=1) as wpool,
                tc.tile_pool(name="xpool", bufs=2) as xpool,
                tc.tile_pool(name="hpool", bufs=1) as hpool,
                tc.tile_pool(name="ps1", bufs=2, space="PSUM") as ps1pool,
                tc.tile_pool(name="ps2", bufs=3, space="PSUM") as ps2pool,
                tc.tile_pool(name="ypool", bufs=4) as ypool,
            ):
                w1s = wpool.tile([P, JBF, F], BF, name="w1s")
                w1s8 = wpool.tile([P, 2, F], F8, name="w1s8")
                w2s = wpool.tile([P, FT, D], BF, name="w2s")
                gs = wpool.tile([P, T // P], F32, name="gs")
                # HAM pre-warm: ~32 zero matmuls keep the PE busy through
                # the startup DMAs so the real matmuls start at 2.4GHz.
                warm_w = wpool.tile([P, P], BF, name="warm_w")
                warm_x = wpool.tile([P, TB], BF, name="warm_x")
                nc.vector.memset(warm_w[:], 0)
                nc.vector.memset(warm_x[:], 0)
                ps_warm = ps1pool.tile([P, TB], F32, name="ps1")
                NWARM = 32
                for i in range(NWARM):
                    nc.tensor.matmul(
                        ps_warm[:, 0:P], warm_w[:], warm_x[:, 0:P],
                        start=(i == 0), stop=(i == NWARM - 1),
                    )

                # critical-path DMA order on the sync queue: block-0
                # activations + first w1 chunk interleaved, then the rest
                # of w1 f-chunk-major, then w2 (first needed ~60us in).
                xs0 = xpool.tile([P, JBF, TB], BF, name="xs")
                xs80 = xpool.tile([P, 2, TB], F8, name="xs8")
                for j in range(JBF):
                    nc.sync.dma_start(xs0[:, j, :], xbf[j, :, 0:TB])
                    nc.sync.dma_start(w1s[:, j, 0:FC], w1bf[0, j, :, :])
                for j in range(2):
                    nc.sync.dma_start(xs80[:, j, :], xf8[j, :, 0:TB])
                    nc.sync.dma_start(w1s8[:, j, 0:FC], w1f8[0, j, :, :])
                for fc in range(1, F // FC):
                    for j in range(JBF):
                        nc.sync.dma_start(
                            w1s[:, j, fc * FC:(fc + 1) * FC], w1bf[fc, j, :, :]
                        )
                    for j in range(2):
                        nc.sync.dma_start(
                            w1s8[:, j, fc * FC:(fc + 1) * FC], w1f8[fc, j, :, :]
                        )
                nc.sync.dma_start(gs[:], g[:])
                for ft in range(FT):
                    nc.sync.dma_start(w2s[:, ft, :], w2bf[ft, :, :])

                for tb in range(NB):
                    if tb == 0:
                        xs, xs8 = xs0, xs80
                    else:
                        xs = xpool.tile([P, JBF, TB], BF, name="xs")
                        xs8 = xpool.tile([P, 2, TB], F8, name="xs8")
                        for j in range(JBF):
                            nc.sync.dma_start(
                                xs[:, j, :], xbf[j, :, tb * TB:(tb + 1) * TB]
                            )
                        for j in range(2):
                            nc.sync.dma_start(
                                xs8[:, j, :], xf8[j, :, tb * TB:(tb + 1) * TB]
                            )
                    # mm1: hT[f, t] = silu(2^-12 * (w1*2^12).T @ xinT)
                    hs = hpool.tile([P, FT, TB], BF, name="hs")
                    for ft in range(FT):
                        ps = ps1pool.tile([P, TB], F32, name="ps1")
                        for j in range(JBF):
                            nc.tensor.matmul(
                                ps[:],
                                w1s[:, j, ft * P:(ft + 1) * P],
                                xs[:, j, :],
                                start=(j == 0),
                                stop=False,
                            )
                        nc.tensor.matmul(
                            ps[:],
                            w1s8[:, 0:2, ft * P:(ft + 1) * P],
                            xs8[:, 0:2, :],
                            start=False,
                            stop=True,
                            perf_mode=DR,
                        )
                        nc.scalar.activation(
                            hs[:, ft, :], ps[:],
                            mybir.ActivationFunctionType.Silu,
                            scale=2.0 ** -12,
                        )
                    # mm2: y[t, d] = hT.T @ (w2*2^12).T, gates carry 2^-12
                    for tt in range(TB // P):
                        col = tb * (TB // P) + tt
                        for dc in range(D // 512):
                            ps2 = ps2pool.tile([P, 512], F32, name="ps2")
                            for ft in range(FT):
                                nc.tensor.matmul(
                                    ps2[:],
                                    hs[:, ft, tt * P:(tt + 1) * P],
                                    w2s[:, ft, dc * 512:(dc + 1) * 512],
                                    start=(ft == 0),
                                    stop=(ft == FT - 1),
                                )
                            last = (tb == NB - 1 and tt == TB // P - 1 and dc == 1)
                            nsplit = 2 if last else 1
                            w = 512 // nsplit
                            for sp in range(nsplit):
                                ys = ypool.tile([P, w], BF, name=f"ys{nsplit}_{sp}")
                                nc.vector.tensor_scalar_mul(
                                    ys[:], ps2[:, sp * w:(sp + 1) * w],
                                    gs[:, col:col + 1],
                                )
                                nc.sync.dma_start(
                                    y[col * P:(col + 1) * P,
                                      dc * 512 + sp * w:dc * 512 + (sp + 1) * w],
                                    ys[:],
                                )
    nc.compile()
    _NC = nc
    _NAMES = (
        xbf.name, xf8.name, w1bf.name, w1f8.name,
        w2bf.name, g.name, y.name,
    )
    return _NC, _NAMES


def _to_bf16(a):
    """Fast f32 -> bf16 with round-to-nearest-even."""
    a = np.ascontiguousarray(a, dtype=np.float32)
    v = a.view(np.uint32)
    r = ((v + np.uint32(0x7FFF) + ((v >> np.uint32(16)) & np.uint32(1)))
         >> np.uint32(16)).astype(np.uint16)
    return r.view(ml_dtypes.bfloat16)


def _to_e4m3(a):
    """f32 -> TRN fp8_e4m3 (IEEE-style; clip to +-240 to stay finite)."""
    a = np.clip(np.ascontiguousarray(a, dtype=np.float32), -240.0, 240.0)
    return a.astype(ml_dtypes.float8_e4m3)


def _routing(x, choice):
    """float64 routing: per (batch, expert) top-K token ids + gates."""
    logits = np.einsum(
        "bsd,ed->bse",
        x.astype(np.float64), choice.astype(np.float64),
        optimize=True,
    )
    m = logits.max(axis=-1, keepdims=True)
    p = np.exp(logits - m)
    probs = p / p.sum(axis=-1, keepdims=True)  # [b, s, e]
    idx = np.empty((B, E, K), dtype=np.int64)
    gates = np.empty((B, E, K), dtype=np.float32)
    for b in range(B):
        for e in range(E):
            pe = probs[b, :, e]
            ii = np.argpartition(-pe, K)[:K]
            ii = np.sort(ii)
            idx[b, e] = ii
            gates[b, e] = pe[ii].astype(np.float32)
    return idx, gates


def kernel(x, choice, w1, w2):
    from concourse.bass_utils import run_bass_kernel_spmd

    x = np.ascontiguousarray(x, dtype=np.float32)
    choice = np.ascontiguousarray(choice, dtype=np.float32)
    w1 = np.ascontiguousarray(w1, dtype=np.float32)
    w2 = np.ascontiguousarray(w2, dtype=np.float32)
    assert x.shape == (B, S, D) and w1.shape == (E, F, D) and w2.shape == (E, D, F)

    nc, names = _build()
    (n_xbf, n_xf8, n_w1bf, n_w1f8, n_w2bf, n_g, n_y) = names

    idx, gates = _routing(x, choice)
    DBF = JBF * P  # 768

    def _prep(e):
        xin = np.empty((T, D), dtype=np.float32)
        for b in range(B):
            xin[b * K:(b + 1) * K] = x[b, idx[b, e], :]
        xinT = np.ascontiguousarray(xin.T)                    # [D, T]
        xbf = np.ascontiguousarray(
            _to_bf16(xinT[:DBF]).reshape(JBF, P, T))
        xf8 = np.ascontiguousarray(
            _to_e4m3(xinT[DBF:]).reshape(2, P, T))
        w1t = np.ascontiguousarray(w1[e].T) * np.float32(4096.0)  # [D, F]
        w1bf = np.ascontiguousarray(
            _to_bf16(w1t[:DBF]).reshape(JBF, P, F // FC, FC)
            .transpose(2, 0, 1, 3))                           # [8, 6, 128, 512]
        w1f8 = np.ascontiguousarray(
            _to_e4m3(w1t[DBF:]).reshape(2, P, F // FC, FC)
            .transpose(2, 0, 1, 3))                           # [8, 2, 128, 512]
        w2bf = np.ascontiguousarray(
            _to_bf16(np.ascontiguousarray(w2[e].T)).reshape(FT, P, D))
        gflat = gates[:, e].reshape(T)
        gcols = np.ascontiguousarray(gflat.reshape(T // P, P).T)  # [P, T//P]
        return {
            n_xbf: xbf, n_xf8: xf8, n_w1bf: w1bf, n_w1f8: w1f8,
            n_w2bf: w2bf, n_g: gcols,
        }

    from concurrent.futures import ThreadPoolExecutor

    with ThreadPoolExecutor(E) as pool:
        in_maps = list(pool.map(_prep, range(E)))

    res = run_bass_kernel_spmd(nc, in_maps, core_ids=list(range(E)))

    out = np.zeros((B, S, D), dtype=np.float32)
    for e in range(E):
        ye = np.asarray(res.results[e][n_y]).astype(np.float32)  # [T, D]
        for b in range(B):
            out[b, idx[b, e], :] += ye[b * K:(b + 1) * K]
    return out


# revision 15
# speedup vs baseline: 1.0483x; 1.0033x over previous
"""MoE expert-choice routing kernel for 8 TRN2 NeuronCores.

Strategy (expert-parallel, one expert per core):
  host: routing in float64 (logits -> softmax -> top-512 tokens per
        (batch, expert)), gather of selected token rows, operand
        pre-transpose + bf16/e4m3 pre-cast.
  device (per core, expert e): y = silu(xin @ w1[e].T) @ w2[e].T scaled
        by the gates; two chained matmuls with the hidden activations
        kept in SBUF.
  host: scatter-add of the 8 per-expert partial outputs (token indices
        are unique within one (batch, expert) pair).

Perf notes vs the plain-bf16 version (469.8us -> 449.6us measured):
  - mm1 does the last 256 of 1024 contraction dims as ONE fp8e4
    DoubleRow matmul (2 fp8 weights per PE cell, contraction 256/pass)
    instead of two bf16 matmuls: 7 instead of 8 PE passes per
    (token-block, ft), ~27us less PE time.  All w1 is pre-scaled by
    2^12 (exact in bf16; lifts the fp8 slice out of e4m3's denormal
    range) and the silu activation descales with scale=2^-12.  Error:
    e4m3 on 25% of mm1's contraction gives 1.85e-2 HW rel err
    (gate 2e-2); mm2 must stay pure bf16 (any more fp8 fails the gate).
  - mm2 runs dc-major so the first half of each output tile drains
    while the second half computes; the very last chunk is split in
    two to pipeline the tail gate-mul with its DMA.
  - y is written as bf16 (host upcasts), halving output DMA.
  Steady-state PE cadence is 216ns/matmul (the bf16 roofline); the
  rest is fixed preamble (~8us), HW power-manager half-speed windows
  (~10us) and DMA-gated startup (~3us).
"""
import sys

if "/opt/trn_rl_repo" not in sys.path:
    sys.path.insert(0, "/opt/trn_rl_repo")

import numpy as np
import ml_dtypes

B = 4          # batch
S = 2048       # tokens per batch (block size)
D = 1024       # d_model
F = 4096       # d_ffn
E = 8          # experts == cores
K = 512        # tokens per (batch, expert)
T = B * K      # 2048 token rows per core
P = 128
TB = 512       # token block in the device kernel
NB = T // TB   # 4
JBF = 6        # bf16 d-blocks of mm1 contraction (dims 0..767)
FT = F // P    # 32
FC = 512       # w1 f-chunk for DMA granularity

_NC = None
_NAMES = None


def _build():
    """Build + compile the per-core Bass program once."""
    global _NC, _NAMES
    if _NC is not None:
        return _NC, _NAMES

    import concourse.mybir as mybir
    import concourse.tile as tile
    from concourse import bacc

    BF = mybir.dt.bfloat16
    F32 = mybir.dt.float32
    F8 = mybir.dt.float8e4
    DR = mybir.MatmulPerfMode.DoubleRow

    nc = bacc.Bacc(None, target_bir_lowering=False)
    with tile.TileContext(nc) as tc:
        with tc.tile_pool(name="dram", bufs=1, space="DRAM") as dram:
            xbf = dram.tile([JBF, P, T], BF, kind="ExternalInput", name="xbf")
            xf8 = dram.tile([2, P, T], F8, kind="ExternalInput", name="xf8")
            w1bf = dram.tile([F // FC, JBF, P, FC], BF, kind="ExternalInput", name="w1bf")
            w1f8 = dram.tile([F // FC, 2, P, FC], F8, kind="ExternalInput", name="w1f8")
            w2bf = dram.tile([FT, P, D], BF, kind="ExternalInput", name="w2bf")
            g = dram.tile([P, T // P], F32, kind="ExternalInput", name="g")
            y = dram.tile([T, D], BF, kind="ExternalOutput", name="y")

            with (
                tc.tile_pool(name="wpool", bufs=1) as wpool,
                tc.tile_pool(name="xpool", bufs=2) as xpool,
                tc.tile_pool(name="hpool", bufs=1) as hpool,
                tc.tile_pool(name="ps1", bufs=2, space="PSUM") as ps1pool,
                tc.tile_pool(name="ps2", bufs=3, space="PSUM") as ps2pool,
                tc.tile_pool(name="ypool", bufs=4) as ypool,
            ):
                w1s = wpool.tile([P, JBF, F], BF, name="w1s")
                w1s8 = wpool.tile([P, 2, F], F8, name="w1s8")
                w2s = wpool.tile([P, FT, D], BF, name="w2s")
                gs = wpool.tile([P, T // P], F32, name="gs")
                # HAM pre-warm: ~32 zero matmuls keep the PE busy through
                # the startup DMAs so the real matmuls start at 2.4GHz.
                warm_w = wpool.tile([P, P], BF, name="warm_w")
                warm_x = wpool.tile([P, TB], BF, name="warm_x")
                nc.vector.memset(warm_w[:], 0)
                nc.vector.memset(warm_x[:], 0)
                ps_warm = ps1pool.tile([P, TB], F32, name="ps1")
                NWARM = 32
                for i in range(NWARM):
                    nc.tensor.matmul(
                        ps_warm[:, 0:P], warm_w[:], warm_x[:, 0:P],
                        start=(i == 0), stop=(i == NWARM - 1),
                    )

                # critical-path DMA order on the sync queue: block-0
                # activations + first w1 chunk interleaved, then the rest
                # of w1 f-chunk-major, then w2 (first needed ~60us in).
                xs0 = xpool.tile([P, JBF, TB], BF, name="xs")
                xs80 = xpool.tile([P, 2, TB], F8, name="xs8")
                for j in range(JBF):
                    nc.sync.dma_start(xs0[:, j, :], xbf[j, :, 0:TB])
                    nc.sync.dma_start(w1s[:, j, 0:FC], w1bf[0, j, :, :])
                for j in range(2):
                    nc.sync.dma_start(xs80[:, j, :], xf8[j, :, 0:TB])
                    nc.sync.dma_start(w1s8[:, j, 0:FC], w1f8[0, j, :, :])
                for fc in range(1, F // FC):
                    for j in range(JBF):
                        nc.sync.dma_start(
                            w1s[:, j, fc * FC:(fc + 1) * FC], w1bf[fc, j, :, :]
                        )
                    for j in range(2):
                        nc.sync.dma_start(
                            w1s8[:, j, fc * FC:(fc + 1) * FC], w1f8[fc, j, :, :]
                        )
                nc.sync.dma_start(gs[:], g[:])
                for ft in range(FT):
                    nc.sync.dma_start(w2s[:, ft, :], w2bf[ft, :, :])

                for tb in range(NB):
                    if tb == 0:
                        xs, xs8 = xs0, xs80
                    else:
                        xs = xpool.tile([P, JBF, TB], BF, name="xs")
                        xs8 = xpool.tile([P, 2, TB], F8, name="xs8")
                        for j in range(JBF):
                            nc.sync.dma_start(
                                xs[:, j, :], xbf[j, :, tb * TB:(tb + 1) * TB]
                            )
                        for j in range(2):
                            nc.sync.dma_start(
                                xs8[:, j, :], xf8[j, :, tb * TB:(tb + 1) * TB]
                            )
                    # mm1: hT[f, t] = silu(2^-12 * (w1*2^12).T @ xinT)
                    hs = hpool.tile([P, FT, TB], BF, name="hs")
                    for ft in range(FT):
                        ps = ps1pool.tile([P, TB], F32, name="ps1")
                        for j in range(JBF):
                            nc.tensor.matmul(
                                ps[:],
                                w1s[:, j, ft * P:(ft + 1) * P],
                                xs[:, j, :],
                                start=(j == 0),
                                stop=False,
                            )
                        nc.tensor.matmul(
                            ps[:],
                            w1s8[:, 0:2, ft * P:(ft + 1) * P],
                            xs8[:, 0:2, :],
                            start=False,
                            stop=True,
                            perf_mode=DR,
                        )
                        nc.scalar.activation(
                            hs[:, ft, :], ps[:],
                            mybir.ActivationFunctionType.Silu,
                            scale=2.0 ** -12,
                        )
                    # mm2: y[t, d] = hT.T @ (w2*2^12).T, gates carry 2^-12
                    for tt in range(TB // P):
                        col = tb * (TB // P) + tt
                        for dc in range(D // 512):
                            ps2 = ps2pool.tile([P, 512], F32, name="ps2")
                            for ft in range(FT):
                                nc.tensor.matmul(
                                    ps2[:],
                                    hs[:, ft, tt * P:(tt + 1) * P],
                                    w2s[:, ft, dc * 512:(dc + 1) * 512],
                                    start=(ft == 0),
                                    stop=(ft == FT - 1),
                                )
                            last = (tb == NB - 1 and tt == TB // P - 1 and dc == 1)
                            nsplit = 2 if last else 1
                            w = 512 // nsplit
                            for sp in range(nsplit):
                                ys = ypool.tile([P, w], BF, name=f"ys{nsplit}_{sp}")
                                nc.vector.tensor_scalar_mul(
                                    ys[:], ps2[:, sp * w:(sp + 1) * w],
                                    gs[:, col:col + 1],
                                )
                                nc.sync.dma_start(
                                    y[col * P:(col + 1) * P,
                                      dc * 512 + sp * w:dc * 512 + (sp + 1) * w],
                                    ys[:],
                                )
    nc.compile()
    _NC = nc
    _NAMES = (
        xbf.name, xf8.name, w1bf.name, w1f8.name,
        w2bf.name, g.name, y.name,
    )
    return _NC, _NAMES


def _to_bf16(a):
    """Fast f32 -> bf16 with round-to-nearest-even."""
    a = np.ascontiguousarray(a, dtype=np.float32)
    v = a.view(np.uint32)
    r = ((v + np.uint32(0x7FFF) + ((v >> np.uint32(16)) & np.uint32(1)))
         >> np.uint32(16)).astype(np.uint16)
    return r.view(ml_dtypes.bfloat16)


def _to_e4m3(a):
    """f32 -> TRN fp8_e4m3 (IEEE-style; clip to +-240 to stay finite)."""
    a = np.clip(np.ascontiguousarray(a, dtype=np.float32), -240.0, 240.0)
    return a.astype(ml_dtypes.float8_e4m3)


def _routing(x, choice):
    """float64 routing: per (batch, expert) top-K token ids + gates."""
    logits = np.einsum(
        "bsd,ed->bse",
        x.astype(np.float64), choice.astype(np.float64),
        optimize=True,
    )
    m = logits.max(axis=-1, keepdims=True)
    p = np.exp(logits - m)
    probs = p / p.sum(axis=-1, keepdims=True)  # [b, s, e]
    idx = np.empty((B, E, K), dtype=np.int64)
    gates = np.empty((B, E, K), dtype=np.float32)
    for b in range(B):
        for e in range(E):
            pe = probs[b, :, e]
            ii = np.argpartition(-pe, K)[:K]
            ii = np.sort(ii)
            idx[b, e] = ii
            gates[b, e] = pe[ii].astype(np.float32)
    return idx, gates


def kernel(x, choice, w1, w2):
    from concourse.bass_utils import run_bass_kernel_spmd

    x = np.ascontiguousarray(x, dtype=np.float32)
    choice = np.ascontiguousarray(choice, dtype=np.float32)
    w1 = np.ascontiguousarray(w1, dtype=np.float32)
    w2 = np.ascontiguousarray(w2, dtype=np.float32)
    assert x.shape == (B, S, D) and w1.shape == (E, F, D) and w2.shape == (E, D, F)

    nc, names = _build()
    (n_xbf, n_xf8, n_w1bf, n_w1f8, n_w2bf, n_g, n_y) = names

    idx, gates = _routing(x, choice)
    DBF = JBF * P  # 768

    def _prep(e):
        xin = np.empty((T, D), dtype=np.float32)
        for b in range(B):
            xin[b * K:(b + 1) * K] = x[b, idx[b, e], :]
        xinT = np.ascontiguousarray(xin.T)                    # [D, T]
        xbf = np.ascontiguousarray(
            _to_bf16(xinT[:DBF]).reshape(JBF, P, T))
        xf8 = np.ascontiguousarray(
            _to_e4m3(xinT[DBF:]).reshape(2, P, T))
        w1t = np.ascontiguousarray(w1[e].T) * np.float32(4096.0)  # [D, F]
        w1bf = np.ascontiguousarray(
            _to_bf16(w1t[:DBF]).reshape(JBF, P, F // FC, FC)
            .transpose(2, 0, 1, 3))                           # [8, 6, 128, 512]
        w1f8 = np.ascontiguousarray(
            _to_e4m3(w1t[DBF:]).reshape(2, P, F // FC, FC)
            .transpose(2, 0, 1, 3))                           # [8, 2, 128, 512]
        w2bf = np.ascontiguousarray(
            _to_bf16(np.ascontiguousarray(w2[e].T)).reshape(FT, P, D))
        gflat = gates[:, e].reshape(T)
        gcols = np.ascontiguousarray(gflat.reshape(T // P, P).T)  # [P, T//P]
        return {
            n_xbf: xbf, n_xf8: xf8, n_w1bf: w1bf, n_w1f8: w1f8,
            n_w2bf: w2bf, n_g: gcols,
        }

    from concurrent.futures import ThreadPoolExecutor

    with ThreadPoolExecutor(E) as pool:
        in_maps = list(pool.map(_prep, range(E)))

    res = run_bass_kernel_spmd(nc, in_maps, core_ids=list(range(E)))

    out = np.zeros((B, S, D), dtype=np.float32)
    for e in range(E):
        ye = np.asarray(res.results[e][n_y]).astype(np.float32)  # [T, D]
        for b in range(B):
            out[b, idx[b, e], :] += ye[b * K:(b + 1) * K]
    return out
